# revision 6
# baseline (speedup 1.0000x reference)
"""Mamba decoder block on 8 Trainium2 NeuronCores.

Sharding: core c in 0..7 -> batch b = c//4, d_inner quarter q = c%4
(512 of 2048 channels). Each core computes the full sequence (L=2048)
for its (b, d-slice). Cross-core dataflow:
  - AllReduce (groups of 4) of the x_proj partial products [96, L]
    (contraction over d_inner is sharded).
  - ReduceScatter (groups of 4) of the out_proj partial [L, 1024];
    core ends up with its L-quarter of the final output.

Selective scan runs on the DVE tensor_tensor_scan instruction
(state = dA*state + dBu along the free/time axis), d-channels on
partitions, one scan per (d-tile, state-index n). exp(A_n * delta) is
computed on the scalar engine with a per-partition scale. B/C rows are
partition-broadcast via DMA from the AllReduce result in DRAM. The
C-contraction over n is a bf16 multiply + pairwise tree sum on DVE.
"""
import sys
import numpy as np

sys.path.insert(0, "/opt/trn_rl_repo")

B, L, D = 2, 2048, 1024
DI, N, DT_RANK, D_CONV = 2048, 16, 64, 4
DS = DI // 4            # d-slice per core
NDT = DS // 128         # 4 d-tiles of 128 channels
TC = 512                # time chunk
NTC = L // TC           # 4 chunks
EPS = 1e-5

_CACHE = {}


def _build_nc():
    import concourse.bacc as bacc
    import concourse.mybir as mybir
    import concourse.tile as tile

    F = mybir.ActivationFunctionType
    A = mybir.AluOpType
    f32, f32r, bf16 = mybir.dt.float32, mybir.dt.float32r, mybir.dt.bfloat16

    nc = bacc.Bacc("TRN2", debug=False, num_devices=8)

    # ---- kernel I/O ----
    xin = nc.dram_tensor("x", [L, D], f32, kind="ExternalInput").ap()
    eye = nc.dram_tensor("eye", [128, 128], f32, kind="ExternalInput").ap()
    wu = nc.dram_tensor("wu", [D, DS], f32, kind="ExternalInput").ap()
    wres = nc.dram_tensor("wres", [D, DS], f32, kind="ExternalInput").ap()
    xpw = nc.dram_tensor("xpw", [DS, DT_RANK + 2 * N], f32, kind="ExternalInput").ap()
    dtw = nc.dram_tensor("dtw", [DT_RANK, DS], f32, kind="ExternalInput").ap()
    dtb = nc.dram_tensor("dtb", [128, NDT], f32, kind="ExternalInput").ap()
    convw = nc.dram_tensor("convw", [128, NDT * D_CONV], f32, kind="ExternalInput").ap()
    convb = nc.dram_tensor("convb", [128, NDT], f32, kind="ExternalInput").ap()
    aneg = nc.dram_tensor("aneg", [128, NDT * N], f32, kind="ExternalInput").ap()
    dpar = nc.dram_tensor("dpar", [128, NDT], f32, kind="ExternalInput").ap()
    wout = nc.dram_tensor("wout", [DS, D], f32, kind="ExternalInput").ap()
    out = nc.dram_tensor("out_chunk", [L // 4, D], f32, kind="ExternalOutput").ap()

    NXP = DT_RANK + 2 * N  # 96

    with tile.TileContext(nc) as tc:
      with tc.tile_pool(name="small", bufs=1) as spool, \
           tc.tile_pool(name="persist", bufs=1) as per, \
           tc.tile_pool(name="scratch", bufs=2) as scr, \
           tc.tile_pool(name="dram", bufs=1, space="DRAM") as dram:

        # small per-partition parameter columns
        dtb_t = spool.tile([128, NDT], f32, tag="dtb")
        nc.sync.dma_start(dtb_t[:], dtb[:])
        convw_t = spool.tile([128, NDT * D_CONV], f32, tag="convw")
        nc.sync.dma_start(convw_t[:], convw[:])
        convb_t = spool.tile([128, NDT], f32, tag="convb")
        nc.sync.dma_start(convb_t[:], convb[:])
        aneg_t = spool.tile([128, NDT * N], f32, tag="aneg")
        nc.sync.dma_start(aneg_t[:], aneg[:])
        dpar_t = spool.tile([128, NDT], f32, tag="dpar")
        nc.sync.dma_start(dpar_t[:], dpar[:])
        wout_k = []
        for k in range(NDT):
            t = spool.tile([128, D], bf16, tag=f"wout{k}", name=f"wout{k}")
            nc.gpsimd.dma_start(t[:], wout[128 * k:128 * (k + 1), :])
            wout_k.append(t)

        # persistent activations (bf16, [128, L] each)
        silu_res = [per.tile([128, L], bf16, tag=f"res{d}", name=f"res{d}")
                    for d in range(NDT)]
        uc = [per.tile([128, L], bf16, tag=f"uc{d}", name=f"uc{d}")
              for d in range(NDT)]
        delta = [per.tile([128, L], bf16, tag=f"dl{d}", name=f"dl{d}")
                 for d in range(NDT)]
        hcarry = [per.tile([128, N], f32, tag=f"hc{d}", name=f"hc{d}")
                  for d in range(NDT)]
        for d in range(NDT):
            nc.vector.memset(hcarry[d][:], 0.0)

        ar_in = dram.tile([NXP, L], f32)
        ar_out = dram.tile([NXP, L], f32)
        rs_in = dram.tile([L, D], f32)
        rs_out = dram.tile([L // 4, D], f32)

        # ================= phases 1-2: norm + in_proj =================
        with tc.tile_pool(name="inproj", bufs=1) as wp2, \
             tc.tile_pool(name="xtiles", bufs=3) as xpl, \
             tc.tile_pool(name="xnTp", bufs=1) as xnp, \
             tc.tile_pool(name="upad", bufs=1) as upool, \
             tc.tile_pool(name="ps_t", bufs=2, space="PSUM") as pst, \
             tc.tile_pool(name="ps_m2", bufs=2, space="PSUM") as psm:

            eye_t = wp2.tile([128, 128], f32, tag="eye")
            nc.sync.dma_start(eye_t[:], eye[:])
            wu_k, wres_k = [], []
            for k in range(8):
                t = wp2.tile([128, DS], f32r, tag=f"wu{k}", name=f"wu{k}")
                nc.gpsimd.dma_start(t[:], wu[128 * k:128 * (k + 1), :])
                wu_k.append(t)
                t2 = wp2.tile([128, DS], f32r, tag=f"wres{k}", name=f"wres{k}")
                nc.gpsimd.dma_start(t2[:], wres[128 * k:128 * (k + 1), :])
                wres_k.append(t2)

            u_pad = [upool.tile([128, L + D_CONV - 1], bf16, tag=f"u{d}",
                                name=f"u{d}") for d in range(NDT)]
            for d in range(NDT):
                nc.vector.memset(u_pad[d][:, 0:D_CONV - 1], 0.0)

            # phase 1: rmsnorm scales (stream x once)
            s_cols = spool.tile([128, 16], f32, tag="scols")
            for i in range(16):
                xt = xpl.tile([128, D], f32, tag="xt")
                nc.sync.dma_start(xt[:], xin[128 * i:128 * (i + 1), :])
                sq = scr.tile([128, D], f32, tag="sq")
                ss = scr.tile([128, 1], f32, tag="ss")
                nc.scalar.activation(sq[:], xt[:], F.Square, accum_out=ss[:])
                ms = scr.tile([128, 1], f32, tag="ms")
                nc.vector.tensor_scalar(ms[:], ss[:], 1.0 / D, EPS, A.mult, A.add)
                rt = scr.tile([128, 1], f32, tag="rt")
                nc.scalar.activation(rt[:], ms[:], F.Sqrt)
                nc.vector.reciprocal(s_cols[:, i:i + 1], rt[:])

            # phase 2: per t-chunk: xn, transpose, in_proj
            for tcb in range(NTC):
                xn_j = []
                for j in range(4):
                    ti = 4 * tcb + j
                    xt = xpl.tile([128, D], f32, tag="xt")
                    nc.sync.dma_start(xt[:], xin[128 * ti:128 * (ti + 1), :])
                    xn = xpl.tile([128, D], f32, tag="xn", bufs=5)
                    nc.scalar.activation(xn[:], xt[:], F.Copy,
                                         scale=s_cols[:, ti:ti + 1])
                    xn_j.append(xn)
                xnT = [xnp.tile([128, TC], f32r, tag=f"xnT{k}", name=f"xnT{k}")
                       for k in range(8)]
                for k in range(8):
                    pt = pst.tile([128, TC], f32, tag="ptr")
                    for j in range(4):
                        nc.tensor.transpose(pt[:, 128 * j:128 * (j + 1)],
                                            xn_j[j][:, 128 * k:128 * (k + 1)],
                                            eye_t[:])
                    nc.scalar.activation(xnT[k][:], pt[:], F.Copy)
                for m in range(NDT):
                    pu = psm.tile([128, TC], f32, tag="pu")
                    for k in range(8):
                        nc.tensor.matmul(pu[:], wu_k[k][:, 128 * m:128 * (m + 1)],
                                         xnT[k][:], start=(k == 0), stop=(k == 7))
                    nc.scalar.activation(
                        u_pad[m][:, D_CONV - 1 + TC * tcb:D_CONV - 1 + TC * (tcb + 1)],
                        pu[:], F.Copy)
                for m in range(NDT):
                    pr = psm.tile([128, TC], f32, tag="pr")
                    for k in range(8):
                        nc.tensor.matmul(pr[:], wres_k[k][:, 128 * m:128 * (m + 1)],
                                         xnT[k][:], start=(k == 0), stop=(k == 7))
                    nc.scalar.activation(silu_res[m][:, TC * tcb:TC * (tcb + 1)],
                                         pr[:], F.Silu)

            # phase 3: conv + silu (uses u_pad; still inside upad scope)
            for d in range(NDT):
                cv = scr.tile([128, L], f32, tag="cv")
                nc.vector.tensor_scalar(cv[:], u_pad[d][:, 0:L],
                                        convw_t[:, D_CONV * d:D_CONV * d + 1],
                                        convb_t[:, d:d + 1], A.mult, A.add)
                for k in range(1, D_CONV):
                    nc.vector.scalar_tensor_tensor(
                        cv[:], u_pad[d][:, k:k + L],
                        convw_t[:, D_CONV * d + k:D_CONV * d + k + 1],
                        cv[:], A.mult, A.add)
                nc.scalar.activation(uc[d][:], cv[:], F.Silu)

        # ================= phase 4: x_proj partial + AllReduce =================
        with tc.tile_pool(name="xproj", bufs=1) as wp3, \
             tc.tile_pool(name="ps_m4", bufs=2, space="PSUM") as psm:
            xpw_k = []
            for k in range(NDT):
                t = wp3.tile([128, NXP], bf16, tag=f"xpw{k}", name=f"xpw{k}")
                nc.gpsimd.dma_start(t[:], xpw[128 * k:128 * (k + 1), :])
                xpw_k.append(t)
            xdp_sb = wp3.tile([NXP, L], f32, tag="xdp")
            for tcb in range(NTC):
                px = psm.tile([NXP, TC], f32, tag="px")
                for k in range(NDT):
                    nc.tensor.matmul(px[:], xpw_k[k][:],
                                     uc[k][:, TC * tcb:TC * (tcb + 1)],
                                     start=(k == 0), stop=(k == NDT - 1))
                nc.scalar.activation(xdp_sb[:, TC * tcb:TC * (tcb + 1)],
                                     px[:], F.Copy)
            nc.sync.dma_start(ar_in[:], xdp_sb[:])
            nc.gpsimd.collective_compute(
                "AllReduce", A.add,
                replica_groups=[[0, 1, 2, 3], [4, 5, 6, 7]],
                ins=[ar_in.opt()], outs=[ar_out.opt()])

        # ================= phase 5: dt_proj + softplus =================
        with tc.tile_pool(name="dtp", bufs=1) as wp4, \
             tc.tile_pool(name="ps_m5", bufs=2, space="PSUM") as psm:
            dtw_t = wp4.tile([DT_RANK, DS], f32r, tag="dtw")
            nc.gpsimd.dma_start(dtw_t[:], dtw[:])
            dpT = wp4.tile([DT_RANK, L], f32r, tag="dpT")
            nc.gpsimd.dma_start(dpT[:], ar_out[0:DT_RANK, :])
            for d in range(NDT):
                for tcb in range(NTC):
                    pd = psm.tile([128, TC], f32, tag="pd")
                    nc.tensor.matmul(pd[:], dtw_t[:, 128 * d:128 * (d + 1)],
                                     dpT[:, TC * tcb:TC * (tcb + 1)],
                                     start=True, stop=True)
                    # softplus(z) = ln(exp(z) + 1); Exp and Ln share a table set
                    ez = scr.tile([128, TC], f32, tag="ez")
                    nc.scalar.activation(ez[:], pd[:], F.Exp,
                                         bias=dtb_t[:, d:d + 1])
                    nc.scalar.activation(delta[d][:, TC * tcb:TC * (tcb + 1)],
                                         ez[:], F.Ln, bias=1.0)

        # ============ phase 6: scan + contraction + out_proj, per t-chunk ======
        with tc.tile_pool(name="bc", bufs=1) as bcp, \
             tc.tile_pool(name="gset", bufs=1) as gset, \
             tc.tile_pool(name="trans", bufs=3) as trans, \
             tc.tile_pool(name="ps_m6", bufs=2, space="PSUM") as psm:
            for tcb in range(NTC):
                t0, t1 = TC * tcb, TC * (tcb + 1)
                bbs, cbs = [], []
                for n in range(N):
                    bb = bcp.tile([128, TC], bf16, tag=f"bb{n}", name=f"bb{n}")
                    nc.gpsimd.dma_start(
                        bb[:], ar_out[DT_RANK + n:DT_RANK + n + 1, t0:t1]
                        .broadcast_to((128, TC)))
                    bbs.append(bb)
                    cb = bcp.tile([128, TC], bf16, tag=f"cb{n}", name=f"cb{n}")
                    nc.gpsimd.dma_start(
                        cb[:], ar_out[DT_RANK + N + n:DT_RANK + N + n + 1, t0:t1]
                        .broadcast_to((128, TC)))
                    cbs.append(cb)
                y_tc = []
                for d in range(NDT):
                    du_t = trans.tile([128, TC], bf16, tag="du")
                    nc.vector.tensor_tensor(du_t[:], delta[d][:, t0:t1],
                                            uc[d][:, t0:t1], A.mult)
                    gs = []
                    for n in range(N):
                        da = trans.tile([128, TC], bf16, tag="da")
                        nc.scalar.activation(
                            da[:], delta[d][:, t0:t1], F.Exp,
                            scale=aneg_t[:, N * d + n:N * d + n + 1])
                        dbu = trans.tile([128, TC], bf16, tag="dbu")
                        nc.vector.tensor_tensor(dbu[:], du_t[:], bbs[n][:], A.mult)
                        h = trans.tile([128, TC], bf16, tag="h")
                        nc.vector.tensor_tensor_scan(
                            h[:], da[:], dbu[:], hcarry[d][:, n:n + 1],
                            A.mult, A.add)
                        nc.vector.tensor_copy(hcarry[d][:, n:n + 1],
                                              h[:, TC - 1:TC])
                        g = gset.tile([128, TC], bf16, tag=f"g{n}", name=f"g{n}")
                        nc.vector.tensor_tensor(g[:], h[:], cbs[n][:], A.mult)
                        gs.append(g)
                    stride = 1
                    while stride < N:
                        for n in range(0, N, 2 * stride):
                            nc.vector.tensor_tensor(gs[n][:], gs[n][:],
                                                    gs[n + stride][:], A.add)
                        stride *= 2
                    tmp = trans.tile([128, TC], bf16, tag="ya")
                    nc.vector.scalar_tensor_tensor(
                        tmp[:], uc[d][:, t0:t1], dpar_t[:, d:d + 1], gs[0][:],
                        A.mult, A.add)
                    yd = trans.tile([128, TC], bf16, tag=f"y{d}", name=f"y{d}")
                    nc.vector.tensor_tensor(yd[:], tmp[:],
                                            silu_res[d][:, t0:t1], A.mult)
                    y_tc.append(yd)
                # out_proj for this t-chunk
                for mt in range(4):
                    tb = 128 * mt
                    for cchunk in range(2):
                        po = psm.tile([128, 512], f32, tag="po")
                        for k in range(NDT):
                            nc.tensor.matmul(
                                po[:], y_tc[k][:, tb:tb + 128],
                                wout_k[k][:, 512 * cchunk:512 * (cchunk + 1)],
                                start=(k == 0), stop=(k == NDT - 1))
                        ot = scr.tile([128, 512], f32, tag="ot")
                        nc.scalar.activation(ot[:], po[:], F.Copy)
                        nc.sync.dma_start(
                            rs_in[t0 + tb:t0 + tb + 128,
                                  512 * cchunk:512 * (cchunk + 1)], ot[:])

        # ================= phase 7: ReduceScatter + store =================
        nc.gpsimd.collective_compute(
            "ReduceScatter", A.add,
            replica_groups=[[0, 1, 2, 3], [4, 5, 6, 7]],
            ins=[rs_in.opt()], outs=[rs_out.opt()])
        nc.sync.dma_start(out[:], rs_out[:])

    nc.finalize()
    return nc


def _get_nc():
    if "nc" not in _CACHE:
        _CACHE["nc"] = _build_nc()
    return _CACHE["nc"]


def _prep_in_maps(x, norm_w, in_proj_w, conv_w, conv_b, x_proj_w, dt_proj_w,
                  dt_proj_b, A_log, D_param, out_proj_w):
    f = np.float32
    wn = (norm_w[:, None] * in_proj_w).astype(f)      # fold norm_w
    a_neg = (-np.exp(A_log)).astype(f)                # [DI, N]
    eye = np.eye(128, dtype=f)
    in_maps = []
    for c in range(8):
        b, q = c // 4, c % 4
        sl = slice(DS * q, DS * (q + 1))
        in_maps.append({
            "x": np.ascontiguousarray(x[b]).astype(f),
            "eye": eye,
            "wu": np.ascontiguousarray(wn[:, sl]),
            "wres": np.ascontiguousarray(wn[:, DI + DS * q: DI + DS * (q + 1)]),
            "xpw": np.ascontiguousarray(x_proj_w[sl, :]).astype(f),
            "dtw": np.ascontiguousarray(dt_proj_w[:, sl]).astype(f),
            "dtb": np.ascontiguousarray(dt_proj_b[sl].reshape(NDT, 128).T).astype(f),
            "convw": np.ascontiguousarray(
                conv_w[sl].reshape(NDT, 128, D_CONV).transpose(1, 0, 2)
                .reshape(128, NDT * D_CONV)).astype(f),
            "convb": np.ascontiguousarray(conv_b[sl].reshape(NDT, 128).T).astype(f),
            "aneg": np.ascontiguousarray(
                a_neg[sl].reshape(NDT, 128, N).transpose(1, 0, 2)
                .reshape(128, NDT * N)).astype(f),
            "dpar": np.ascontiguousarray(D_param[sl].reshape(NDT, 128).T).astype(f),
            "wout": np.ascontiguousarray(out_proj_w[sl, :]).astype(f),
        })
    return in_maps


def kernel(x, norm_w, in_proj_w, conv_w, conv_b, x_proj_w, dt_proj_w,
           dt_proj_b, A_log, D_param, out_proj_w, _trace=False):
    from concourse.bass_utils import run_bass_kernel_spmd

    nc = _get_nc()
    in_maps = _prep_in_maps(
        np.asarray(x), np.asarray(norm_w), np.asarray(in_proj_w),
        np.asarray(conv_w), np.asarray(conv_b), np.asarray(x_proj_w),
        np.asarray(dt_proj_w), np.asarray(dt_proj_b), np.asarray(A_log),
        np.asarray(D_param), np.asarray(out_proj_w))
    res = run_bass_kernel_spmd(nc, in_maps, core_ids=list(range(8)),
                               trace=_trace)
    _CACHE["last_result"] = res
    out = np.empty((B, L, D), np.float32)
    for c in range(8):
        b, q = c // 4, c % 4
        out[b, (L // 4) * q:(L // 4) * (q + 1), :] = res.results[c]["out_chunk"]
    return out
